# revision 1
# baseline (speedup 1.0000x reference)
import numpy as np
import jax
import jax.numpy as jnp
from functools import partial

# Problem constants (hardcoded per harness contract)
B, N, K, H = 1, 8192, 32, 16
NCORES = 8
NS = N // NCORES  # nodes per core

FIBER_IN = (16, 8)
NC_OUT = {'00': 8, '10': 8, '01': 4, '11': 4}
PAIRS = (('00', 16, 8, 1), ('10', 8, 8, 1), ('01', 16, 4, 1), ('11', 8, 4, 3))


def _silu(x):
    return x * jax.nn.sigmoid(x)


def _ln(x, g):
    mu = x.mean(-1, keepdims=True)
    var = ((x - mu) ** 2).mean(-1, keepdims=True)
    return (x - mu) * jax.lax.rsqrt(var + 1e-5) * g


def _radial(feat, p):
    w1, b1, g1, w2, b2, g2, w3, b3 = p
    h = _ln(_silu(feat @ w1 + b1), g1)
    h = _ln(_silu(h @ w2 + b2), g2)
    return h @ w3 + b3


def _lin(x, w):
    return jnp.einsum('ndm,de->nem', x, w)


def _shard_forward(x0f, x1f, rel_dist, b00, b10, b01, b11,
                   w_xi0, w_xi1, w_xj0, w_xj1, rps_flat,
                   w_out0, w_out1, w_si0, w_si1, nbr_idx, nbr_mask):
    """Per-core computation. x0f/x1f are FULL replicated node tables [N,d,m];
    everything else is the node shard [NS,...]. Batch dim dropped."""
    rps = {}
    names = ('00', '10', '01', '11')
    for i, nm in enumerate(names):
        rps[nm] = tuple(rps_flat[i * 8: i * 8 + 8])
    bases = {'00': b00, '10': b10, '01': b01, '11': b11}
    inp_full = {0: x0f, 1: x1f}

    # xj over FULL table (for gathers); xi over the local shard rows
    ns = nbr_idx.shape[0]
    xj = {d: _lin(inp_full[d], (w_xj0, w_xj1)[d]) for d in (0, 1)}
    # local node rows of the shard: the shard owns rows [off, off+NS) but we
    # simply gather xi from the full table using local indices too
    xi = {d: _lin(inp_full[d], (w_xi0, w_xi1)[d]) for d in (0, 1)}

    mask = nbr_mask.astype(jnp.float32)  # [ns,k]
    denom = jnp.maximum(mask.sum(-1), 1.0)[..., None, None]  # [ns,1,1]

    outs = {}
    self_rows = jax.lax.iota(jnp.int32, ns) + nbr_idx.shape[-1] * 0  # placeholder
    for do in (0, 1):
        chunks = []
        for di in (0, 1):
            pkey = str(di) + str(do)
            ci, co = FIBER_IN[di], NC_OUT[pkey]
            nf = 2 * min(di, do) + 1
            xg = xj[di][nbr_idx] + xi[di][None].swapaxes(0, 1) * 0  # placeholder, replaced below
            chunks.append(xg)
        outs[do] = chunks
    # (real computation below in caller-specialized function)
    raise RuntimeError("unused")


@partial(jax.pmap, axis_name='x',
         in_axes=(None, None, 0, 0, 0, 0, 0,
                  None, None, None, None, None,
                  None, None, None, None, 0, 0, 0))
def _kernel_pmap(x0f, x1f, rel_dist, b00, b10, b01, b11,
                 w_xi0, w_xi1, w_xj0, w_xj1, rps_flat,
                 w_out0, w_out1, w_si0, w_si1, nbr_idx, nbr_mask, self_idx):
    names = ('00', '10', '01', '11')
    rps = {nm: tuple(rps_flat[i * 8: i * 8 + 8]) for i, nm in enumerate(names)}
    bases = {'00': b00, '10': b10, '01': b01, '11': b11}
    inp_full = {0: x0f, 1: x1f}

    xj = {d: _lin(inp_full[d], (w_xj0, w_xj1)[d]) for d in (0, 1)}
    xi_full = {d: _lin(inp_full[d], (w_xi0, w_xi1)[d]) for d in (0, 1)}
    # local shard's xi rows
    xi = {d: xi_full[d][self_idx] for d in (0, 1)}
    x_local = {d: inp_full[d][self_idx] for d in (0, 1)}

    ns, k = nbr_idx.shape
    mask = nbr_mask.astype(jnp.float32)  # [ns,k]
    denom = jnp.maximum(mask.sum(-1), 1.0)[..., None, None]  # [ns,1,1]

    outs = {}
    for do in (0, 1):
        chunks = []
        for di in (0, 1):
            pkey = str(di) + str(do)
            ci, co = FIBER_IN[di], NC_OUT[pkey]
            nf = 2 * min(di, do) + 1
            xg = xj[di][nbr_idx] + xi[di][:, None]  # [ns,k,ci,mi]
            R = _radial(rel_dist, rps[pkey]).reshape(ns, k, co, ci, nf)
            Bt = bases[pkey].reshape(ns, k, 2 * do + 1, 2 * di + 1, nf)
            tmp = jnp.einsum('nkpqf,nkiq->nkpfi', Bt, xg)
            chunks.append(jnp.einsum('nkoif,nkpfi->nkop', R, tmp))
        out = jnp.concatenate(chunks, axis=2)  # [ns,k,dim_out,mo]
        outs[do] = (out * mask[..., None, None]).sum(1) / denom

    o0 = _lin(outs[0], w_out0) + _lin(x_local[0], w_si0)
    o1 = _lin(outs[1], w_out1) + _lin(x_local[1], w_si1)
    return o0, o1


def kernel(x0, x1, rel_dist, basis_0_0, basis_1_0, basis_0_1, basis_1_1,
           w_xi0, w_xi1, w_xj0, w_xj1,
           rp00_w1, rp00_b1, rp00_g1, rp00_w2, rp00_b2, rp00_g2, rp00_w3, rp00_b3,
           rp10_w1, rp10_b1, rp10_g1, rp10_w2, rp10_b2, rp10_g2, rp10_w3, rp10_b3,
           rp01_w1, rp01_b1, rp01_g1, rp01_w2, rp01_b2, rp01_g2, rp01_w3, rp01_b3,
           rp11_w1, rp11_b1, rp11_g1, rp11_w2, rp11_b2, rp11_g2, rp11_w3, rp11_b3,
           w_out0, w_out1, w_si0, w_si1, neighbor_indices, neighbor_mask):
    f32 = np.float32
    x0 = np.asarray(x0, f32); x1 = np.asarray(x1, f32)

    # node-shard [8, NS, ...] views of per-pair data
    def shard(a):
        a = np.asarray(a)
        return a.reshape((NCORES, NS) + a.shape[2:])

    rel_s = shard(rel_dist.astype(f32))
    b00_s = shard(np.asarray(basis_0_0, f32).reshape(1, N, K, 1, 1, 1))
    b10_s = shard(np.asarray(basis_1_0, f32).reshape(1, N, K, 1, 3, 1))
    b01_s = shard(np.asarray(basis_0_1, f32).reshape(1, N, K, 3, 1, 1))
    b11_s = shard(np.asarray(basis_1_1, f32).reshape(1, N, K, 3, 3, 3))
    idx_s = shard(np.asarray(neighbor_indices, np.int32))
    msk_s = shard(np.asarray(neighbor_mask))
    self_idx = np.arange(N, dtype=np.int32).reshape(NCORES, NS)

    rps_flat = np.stack([
        np.zeros((H, max(H, 1)), f32)  # placeholder; replaced below per-entry shapes differ
    ]) if False else None

    # radial params: shapes differ across entries -> pad into a uniform stack
    # easier: pass as a flat tuple of arrays via closure-free pmap -> we instead
    # broadcast each param individually. Build list in fixed order:
    rp_list = [rp00_w1, rp00_b1, rp00_g1, rp00_w2, rp00_b2, rp00_g2, rp00_w3, rp00_b3,
               rp10_w1, rp10_b1, rp10_g1, rp10_w2, rp10_b2, rp10_g2, rp10_w3, rp10_b3,
               rp01_w1, rp01_b1, rp01_g1, rp01_w2, rp01_b2, rp01_g2, rp01_w3, rp01_b3,
               rp11_w1, rp11_b1, rp11_g1, rp11_w2, rp11_b2, rp11_g2, rp11_w3, rp11_b3]
    rp_list = [np.asarray(a, f32) for a in rp_list]

    o0, o1 = _kernel_pmap(
        x0[0], x1[0],
        rel_s[:, :, :, 0] if False else rel_s,  # keep [8,NS,K,1]
        b00_s, b10_s, b01_s, b11_s,
        np.asarray(w_xi0, f32), np.asarray(w_xi1, f32),
        np.asarray(w_xj0, f32), np.asarray(w_xj1, f32),
        tuple(rp_list),
        np.asarray(w_out0, f32), np.asarray(w_out1, f32),
        np.asarray(w_si0, f32), np.asarray(w_si1, f32),
        idx_s, msk_s, self_idx)

    o0 = np.asarray(o0).reshape(1, N, 16, 1).astype(f32)
    o1 = np.asarray(o1).reshape(1, N, 8, 3).astype(f32)
    return o0, o1


# revision 2
# speedup vs baseline: 3.7429x; 3.7429x over previous
import numpy as np
import jax
import jax.numpy as jnp
from functools import partial

# Problem constants (hardcoded per harness contract)
B, N, K, H = 1, 8192, 32, 16
NCORES = 8
NS = N // NCORES  # nodes per core

FIBER_IN = (16, 8)
NC_OUT = {'00': 8, '10': 8, '01': 4, '11': 4}
PAIRS = (('00', 16, 8, 1), ('10', 8, 8, 1), ('01', 16, 4, 1), ('11', 8, 4, 3))


def _silu(x):
    return x * jax.nn.sigmoid(x)


def _ln(x, g):
    mu = x.mean(-1, keepdims=True)
    var = ((x - mu) ** 2).mean(-1, keepdims=True)
    return (x - mu) * jax.lax.rsqrt(var + 1e-5) * g


def _radial(feat, p):
    w1, b1, g1, w2, b2, g2, w3, b3 = p
    h = _ln(_silu(feat @ w1 + b1), g1)
    h = _ln(_silu(h @ w2 + b2), g2)
    return h @ w3 + b3


def _lin(x, w):
    return jnp.einsum('ndm,de->nem', x, w)


@partial(jax.pmap, axis_name='x',
         in_axes=(None, None, 0, 0, 0, 0, 0,
                  None, None, None, None, None,
                  None, None, None, None, 0, 0, 0))
def _kernel_pmap(x0f, x1f, rel_dist, b00, b10, b01, b11,
                 w_xi0, w_xi1, w_xj0, w_xj1, rps_flat,
                 w_out0, w_out1, w_si0, w_si1, nbr_idx, nbr_mask, self_idx):
    names = ('00', '10', '01', '11')
    rps = {nm: tuple(rps_flat[i * 8: i * 8 + 8]) for i, nm in enumerate(names)}
    bases = {'00': b00, '10': b10, '01': b01, '11': b11}
    inp_full = {0: x0f, 1: x1f}

    xj = {d: _lin(inp_full[d], (w_xj0, w_xj1)[d]) for d in (0, 1)}
    xi_full = {d: _lin(inp_full[d], (w_xi0, w_xi1)[d]) for d in (0, 1)}
    # local shard's xi rows
    xi = {d: xi_full[d][self_idx] for d in (0, 1)}
    x_local = {d: inp_full[d][self_idx] for d in (0, 1)}

    ns, k = nbr_idx.shape
    mask = nbr_mask.astype(jnp.float32)  # [ns,k]
    denom = jnp.maximum(mask.sum(-1), 1.0)[..., None, None]  # [ns,1,1]

    outs = {}
    for do in (0, 1):
        chunks = []
        for di in (0, 1):
            pkey = str(di) + str(do)
            ci, co = FIBER_IN[di], NC_OUT[pkey]
            nf = 2 * min(di, do) + 1
            xg = xj[di][nbr_idx] + xi[di][:, None]  # [ns,k,ci,mi]
            R = _radial(rel_dist, rps[pkey]).reshape(ns, k, co, ci, nf)
            Bt = bases[pkey].reshape(ns, k, 2 * do + 1, 2 * di + 1, nf)
            tmp = jnp.einsum('nkpqf,nkiq->nkpfi', Bt, xg)
            chunks.append(jnp.einsum('nkoif,nkpfi->nkop', R, tmp))
        out = jnp.concatenate(chunks, axis=2)  # [ns,k,dim_out,mo]
        outs[do] = (out * mask[..., None, None]).sum(1) / denom

    o0 = _lin(outs[0], w_out0) + _lin(x_local[0], w_si0)
    o1 = _lin(outs[1], w_out1) + _lin(x_local[1], w_si1)
    return o0, o1


def kernel(x0, x1, rel_dist, basis_0_0, basis_1_0, basis_0_1, basis_1_1,
           w_xi0, w_xi1, w_xj0, w_xj1,
           rp00_w1, rp00_b1, rp00_g1, rp00_w2, rp00_b2, rp00_g2, rp00_w3, rp00_b3,
           rp10_w1, rp10_b1, rp10_g1, rp10_w2, rp10_b2, rp10_g2, rp10_w3, rp10_b3,
           rp01_w1, rp01_b1, rp01_g1, rp01_w2, rp01_b2, rp01_g2, rp01_w3, rp01_b3,
           rp11_w1, rp11_b1, rp11_g1, rp11_w2, rp11_b2, rp11_g2, rp11_w3, rp11_b3,
           w_out0, w_out1, w_si0, w_si1, neighbor_indices, neighbor_mask):
    f32 = np.float32
    x0 = np.asarray(x0, f32); x1 = np.asarray(x1, f32)

    # node-shard [8, NS, ...] views of per-pair data
    def shard(a):
        a = np.asarray(a)
        return a.reshape((NCORES, NS) + a.shape[2:])

    rel_s = shard(rel_dist.astype(f32))
    b00_s = shard(np.asarray(basis_0_0, f32).reshape(1, N, K, 1, 1, 1))
    b10_s = shard(np.asarray(basis_1_0, f32).reshape(1, N, K, 1, 3, 1))
    b01_s = shard(np.asarray(basis_0_1, f32).reshape(1, N, K, 3, 1, 1))
    b11_s = shard(np.asarray(basis_1_1, f32).reshape(1, N, K, 3, 3, 3))
    idx_s = shard(np.asarray(neighbor_indices, np.int32))
    msk_s = shard(np.asarray(neighbor_mask))
    self_idx = np.arange(N, dtype=np.int32).reshape(NCORES, NS)

    rps_flat = np.stack([
        np.zeros((H, max(H, 1)), f32)  # placeholder; replaced below per-entry shapes differ
    ]) if False else None

    # radial params: shapes differ across entries -> pad into a uniform stack
    # easier: pass as a flat tuple of arrays via closure-free pmap -> we instead
    # broadcast each param individually. Build list in fixed order:
    rp_list = [rp00_w1, rp00_b1, rp00_g1, rp00_w2, rp00_b2, rp00_g2, rp00_w3, rp00_b3,
               rp10_w1, rp10_b1, rp10_g1, rp10_w2, rp10_b2, rp10_g2, rp10_w3, rp10_b3,
               rp01_w1, rp01_b1, rp01_g1, rp01_w2, rp01_b2, rp01_g2, rp01_w3, rp01_b3,
               rp11_w1, rp11_b1, rp11_g1, rp11_w2, rp11_b2, rp11_g2, rp11_w3, rp11_b3]
    rp_list = [np.asarray(a, f32) for a in rp_list]

    o0, o1 = _kernel_pmap(
        x0[0], x1[0],
        rel_s[:, :, :, 0] if False else rel_s,  # keep [8,NS,K,1]
        b00_s, b10_s, b01_s, b11_s,
        np.asarray(w_xi0, f32), np.asarray(w_xi1, f32),
        np.asarray(w_xj0, f32), np.asarray(w_xj1, f32),
        tuple(rp_list),
        np.asarray(w_out0, f32), np.asarray(w_out1, f32),
        np.asarray(w_si0, f32), np.asarray(w_si1, f32),
        idx_s, msk_s, self_idx)

    o0 = np.asarray(o0).reshape(1, N, 16, 1).astype(f32)
    o1 = np.asarray(o1).reshape(1, N, 8, 3).astype(f32)
    return o0, o1
